# revision 34
# baseline (speedup 1.0000x reference)
"""Trainium2 Bass kernel for a 2-layer tanh RNN (H=512) over T=32768, batch 1.

Strategy: chunked sequence parallelism with warmup, in bf16. The RNN map
h_t = tanh(pre_t + W_hh h_{t-1}) is contractive (spectral radius ~0.64), so
a chunk recurrence started W steps early from a zero state converges to the
true trajectory; the bf16 quantization noise (~3e-3 rel) dominates the
warmup residual, so W=6 suffices. We split T into 1024 chunks of L=32; each
of the 8 cores advances its 130 chunk-states (128 real + 2 warmup heads) as
one batched recurrence: each step is a [512,512] x [512,130] matmul block
(16 PE tiles, bf16 + automatic fast-weight-load) plus 4 fused input-
injection matmuls and 2 tanh ops. Chunk 0's state is reset to the true h0
after its warmup, making it exact up to bf16 rounding.

Phases per core (fully SPMD, no cross-core communication):
  A) layer-0 batched recurrence (input proj fused as matmuls from x^T;
     w0x zero-padded to K=128 so its weight loads get FWL)
  B) batched GEMM pre1 = W_ih1 @ h1 + bias over the core's time range,
     two timesteps per matmul (adjacent t-slices are contiguous in the
     kept tiles); bias added on the Vector engine (psum f32 -> bf16 SBUF)
  C) layer-1 batched recurrence (pre1 injected via identity matmuls), with
     the output GEMM out = W_fc @ h2 + b_fc emitted per 4 kept timesteps
     (one [4,512] psum group) as soon as they land; the tile scheduler
     interleaves B/D matmuls into recurrence dependency stalls.

bf16 rationale: fp32r needs a >=256-wide moving operand for full rate, which
forces L=16 and a 2x warmup overhead (W=16). bf16 runs 1 col/cycle at any
width and halves weight-load time (automatic FWL), so L=32/W=6 cuts the
per-timestep PE columns by ~30% and the DMA bytes by 2x. PSUM layout: each
of the 8 banks holds at most one open accumulation group, so each m-group
gets its own 512-col bank slice ([128,1024] tiles); phase B/D psum shares
the psA/psB slot rotation. Measured: 256us (fp32r baseline) -> ~168us,
end-to-end rel error 6.6e-3 (vs 2e-2 budget).
"""

import numpy as np
import ml_dtypes

import concourse.bass as bass
import concourse.mybir as mybir
from concourse.tile import TileContext
from concourse.bass_utils import run_bass_kernel_spmd

# ---------------------------------------------------------------- constants
T = 32768
H = 512
IN = 40
NC = 8
L = 32          # chunk length
W = 6           # warmup steps
EX = 2          # extra head chunks per core (W <= EX*L)
TC = T // NC    # timesteps per core
B = TC // L     # real chunks per core
BT = B + EX     # batched chunks per core (layer 0)
XW = BT + 2     # x^T slab width
S = L + W       # recurrence steps per layer
BF16 = mybir.dt.bfloat16
F32 = mybir.dt.float32
ACT = mybir.ActivationFunctionType
NPBF16 = ml_dtypes.bfloat16

TRACE = False        # set by test harness for profiled runs
DEBUG = False        # add intermediate-state DRAM dumps to the program
SPLIT_WAITS = True   # walrus maxw=1 workaround (disable for CoreSim)
LAST_RESULT = None

_ctr = [0]


def _split_sync_waits(nc, maxw=1):
    """walrus in this container encodes at most `maxw` sem-waits per
    instruction; move excess waits onto same-engine NOPs inserted right
    before the instruction (engine program order keeps semantics)."""
    for f in nc.m.functions:
        for bb in f.blocks:
            il = bb.instructions
            targets = []
            for idx, inst in enumerate(il):
                si = inst.sync_info
                if si is not None and si.on_wait is not None and len(si.on_wait) > maxw:
                    targets.append(idx)
            for idx in reversed(targets):
                inst = il[idx]
                si = inst.sync_info
                waits = list(si.on_wait)
                excess = waits[:-maxw]
                inst.sync_info = mybir.SyncInfo(
                    on_wait=waits[-maxw:], on_update=list(si.on_update)
                )
                nops = []
                for j in range(0, len(excess), maxw):
                    _ctr[0] += 1
                    nop = mybir.InstNoOp(name=f"wsplit_nop_{_ctr[0]}")
                    nop.engine = inst.engine
                    nop.sync_info = mybir.SyncInfo(
                        on_wait=excess[j : j + maxw], on_update=[]
                    )
                    nops.append(nop)
                for k, nop in enumerate(nops):
                    il.insert(idx + k, nop)
    return nc


def _recurrence(nc, psp, whh, inject, kept, scr, n, reset, post=None):
    """S batched recurrence steps for one layer.

    kept: (keptA, keptB) flat tiles [128, 2*L*n], col = kh*(L*n) + t*n + b.
    scr:  (scrA, scrB) flat tiles [128, 2*2*n], col = kh*(2*n) + c*n + b.
    inject(m, tau, ps_ap, stop): opens the psum group for output block m.
    reset(c): chunk-0 h0 override hook on scratch ping-pong column c.
    post(t): called after the ACTs of step W+t (t in [0, L)).
    """
    def h_src(k, tp):
        """Contiguous moving AP [128, n] for k-block state after step tp."""
        kh = k % 2
        if tp >= W:
            return kept[k // 2][:, kh * L * n + (tp - W) * n :][:, :n]
        return scr[k // 2][:, kh * 2 * n + (tp % 2) * n :][:, :n]

    def h_dst(a, tau):
        """ACT dst AP [128, 2, n] for half a at step tau."""
        if tau >= W:
            c = tau - W
            return kept[a][:].rearrange("p (k t b) -> p k t b", k=2, t=L)[:, :, c, :]
        c = tau % 2
        return scr[a][:].rearrange("p (k t b) -> p k t b", k=2, t=2)[:, :, c, :]

    for tau in range(S):
        # [128,1024] = 2 PSUM banks; each m-group gets its own bank (one
        # open accumulation group per bank is a hardware constraint).
        psA = psp.tile([128, 1024], F32, name=f"psA_{_ctr[0]}_{tau}", tag="psA")
        psB = psp.tile([128, 1024], F32, name=f"psB_{_ctr[0]}_{tau}", tag="psB")
        ps = (psA, psB)

        def mm_ap(m):
            return ps[m // 2][:, 512 * (m % 2) : 512 * (m % 2) + n]

        # Emit ALL writers of the psA half (m0,m1) before any psB-only work,
        # so psA closes at 10/20 matmuls and the psA-close -> tanh ->
        # next-step-gate chain hides entirely under the psB half's matmuls
        # (the per-step chain is what paces the recurrence).
        for mh in range(2):
            ms = (2 * mh, 2 * mh + 1)
            for m in ms:
                inject(m, tau, mm_ap(m), tau == 0)
            if tau > 0:
                for k in (0, 1):
                    for m in ms:
                        nc.tensor.matmul(
                            mm_ap(m),
                            whh[:, 512 * k + 128 * m : 512 * k + 128 * m + 128],
                            h_src(k, tau - 1),
                            start=False, stop=False,
                        )
                for m in ms:
                    for k in (2, 3):
                        nc.tensor.matmul(
                            mm_ap(m),
                            whh[:, 512 * k + 128 * m : 512 * k + 128 * m + 128],
                            h_src(k, tau - 1),
                            start=False, stop=(k == 3),
                        )
            nc.scalar.activation(
                h_dst(mh, tau),
                ps[mh][:].rearrange("p (m c) -> p m c", m=2)[:, :, :n],
                ACT.Tanh,
            )
        if reset is not None and tau == W - 1:
            reset((W - 1) % 2)
        if post is not None and tau >= W:
            post(tau - W)


def _build_program():
    nc = bass.Bass()
    xt_d = nc.dram_tensor("xt", [128, L * XW], BF16, kind="ExternalInput")
    w0x_d = nc.dram_tensor("w0x", [128, 512], BF16, kind="ExternalInput")
    whh0_d = nc.dram_tensor("whh0", [128, 2048], BF16, kind="ExternalInput")
    whh1_d = nc.dram_tensor("whh1", [128, 2048], BF16, kind="ExternalInput")
    wih1_d = nc.dram_tensor("wih1", [128, 2048], BF16, kind="ExternalInput")
    bias1_d = nc.dram_tensor("bias1", [128, 4], F32, kind="ExternalInput")
    wfc_d = nc.dram_tensor("wfc", [128, 16], BF16, kind="ExternalInput")
    bfc_d = nc.dram_tensor("bfc", [3, 1], F32, kind="ExternalInput")
    eye_d = nc.dram_tensor("eye", [128, 128], BF16, kind="ExternalInput")
    hcm_d = nc.dram_tensor("hcm", [128, 32], BF16, kind="ExternalInput")
    out_d = nc.dram_tensor("out", [3, L, B], F32, kind="ExternalOutput")
    if DEBUG:
        dk1A_d = nc.dram_tensor("dk1A", [128, 2 * L * BT], BF16, kind="ExternalOutput")
        dk1B_d = nc.dram_tensor("dk1B", [128, 2 * L * BT], BF16, kind="ExternalOutput")
        dpre_d = nc.dram_tensor("dpre", [128, 4 * L * BT], BF16, kind="ExternalOutput")
        dk2A_d = nc.dram_tensor("dk2A", [128, 2 * L * B], BF16, kind="ExternalOutput")
        dk2B_d = nc.dram_tensor("dk2B", [128, 2 * L * B], BF16, kind="ExternalOutput")

    import contextlib
    with TileContext(nc) as tc, contextlib.ExitStack() as ctx:
        const = ctx.enter_context(tc.tile_pool(name="const", bufs=1))
        big = ctx.enter_context(tc.tile_pool(name="big", bufs=1))
        outp = ctx.enter_context(tc.tile_pool(name="outp", bufs=2))
        psp = ctx.enter_context(tc.tile_pool(name="psp", bufs=2, space="PSUM"))

        # Critical-path DMAs (gate the first recurrence steps). Each
        # dma_start costs ~600ns of issuing-engine time, so x streams in as
        # 4 big blocks, not 32 slabs. The scalar queue is kept nearly free:
        # ScE runs the per-step tanh.
        xt = const.tile([128, L * XW], BF16)
        w0x = const.tile([128, 512], BF16)
        whh0 = const.tile([128, 2048], BF16)
        hcm = const.tile([128, 32], BF16)
        # block 0 is tiny (2 slabs) so the first step's gate clears fast;
        # whh0 (needed at step 1) is split 4 ways across the three queues
        nc.sync.dma_start(xt[:, : 2 * XW], xt_d[:, : 2 * XW])
        nc.scalar.dma_start(w0x[:], w0x_d[:])
        nc.gpsimd.dma_start(whh0[:, :512], whh0_d[:, :512])
        nc.gpsimd.dma_start(whh0[:, 512:1024], whh0_d[:, 512:1024])
        nc.sync.dma_start(whh0[:, 1024:1536], whh0_d[:, 1024:1536])
        nc.scalar.dma_start(whh0[:, 1536:], whh0_d[:, 1536:])
        nc.scalar.dma_start(hcm[:], hcm_d[:])
        h0r = hcm[:, 0:16]
        cm = hcm[:, 16:32]
        # remaining x blocks stream in behind the recurrence
        qs = (nc.sync, nc.gpsimd, nc.sync)
        for blk, (u0, u1) in enumerate([(2, 12), (12, 22), (22, 32)]):
            qs[blk].dma_start(xt[:, u0 * XW : u1 * XW],
                              xt_d[:, u0 * XW : u1 * XW])
        whh1 = const.tile([128, 2048], BF16)
        wih1 = const.tile([128, 2048], BF16)
        bias1 = const.tile([128, 4], F32)
        wfc = const.tile([128, 16], BF16)
        bfc = const.tile([3, 1], F32)
        eye = const.tile([128, 128], BF16)

        # -------------------------------------------------------- phase A
        k1A = big.tile([128, 2 * L * BT], BF16, tag="kA")
        k1B = big.tile([128, 2 * L * BT], BF16, tag="kB")
        s1A = big.tile([128, 2 * 2 * BT], BF16, tag="sA")
        s1B = big.tile([128, 2 * 2 * BT], BF16, tag="sB")

        def inj0(m, tau, ps_ap, stop):
            # w0x is zero-padded to K=128 so the weight load gets FWL
            q, u = tau // L, tau % L
            nc.tensor.matmul(
                ps_ap, w0x[:, 128 * m : 128 * m + 128],
                xt[:, u * XW + q :][:, :BT],
                start=True, stop=stop,
            )

        def reset0(c):
            for scr, off in ((s1A, 0), (s1B, 8)):
                ap = scr[:, c * BT + EX : c * BT + EX + 2 * BT + 1 : 2 * BT]
                nc.vector.tensor_tensor(ap, ap, cm[:, off : off + 2],
                                        mybir.AluOpType.mult)
                nc.vector.tensor_tensor(ap, ap, h0r[:, off : off + 2],
                                        mybir.AluOpType.add)

        _recurrence(nc, psp, whh0, inj0, (k1A, k1B), (s1A, s1B), BT, reset0)

        # later-phase weights: emitted after phase A so they don't gate its
        # start; the DMA queues drain them while the PE runs layer 0.
        nc.gpsimd.dma_start(wih1[:], wih1_d[:])
        nc.sync.dma_start(whh1[:], whh1_d[:])
        nc.sync.dma_start(bias1[:], bias1_d[:])
        nc.gpsimd.dma_start(wfc[:], wfc_d[:])
        nc.gpsimd.dma_start(bfc[:], bfc_d[:])
        nc.sync.dma_start(eye[:], eye_d[:])

        # -------------------------------------------------------- phase B
        # pre1 step-major: col = m*(L*BT) + t*BT + b  (same indexing as h1
        # kept). Emitted in layer-1 consumption order: warmup needs t=24..31
        # first. Bias is added on the Vector engine (psum f32 -> bf16 SBUF).
        pre1s = big.tile([128, 4 * L * BT], BF16, tag="pre1")
        ts = list(range(L - W, L)) + list(range(L - W))
        # batch adjacent-t pairs (contiguous in the kept tiles -> one 2*BT-col
        # matmul); pair within the consumption-order list
        tps = []
        seen = set()
        for t in ts:
            t0 = t - t % 2
            if t0 not in seen:
                seen.add(t0)
                tps.append(t0)
        for t0 in tps:
            for mp in range(2):
                pg = psp.tile([128, 1024], F32, name=f"pg_{t0}_{mp}", tag="psB")
                for mi in range(2):
                    m = 2 * mp + mi
                    for k in range(4):
                        kt = k1A if k < 2 else k1B
                        nc.tensor.matmul(
                            pg[:, 512 * mi : 512 * mi + 2 * BT],
                            wih1[:, 512 * k + 128 * m : 512 * k + 128 * m + 128],
                            kt[:, (k % 2) * L * BT + t0 * BT :][:, : 2 * BT],
                            start=(k == 0), stop=(k == 3),
                        )
                for mi in range(2):
                    m = 2 * mp + mi
                    nc.vector.tensor_scalar_add(
                        pre1s[:, m * L * BT + t0 * BT :][:, : 2 * BT],
                        pg[:, 512 * mi : 512 * mi + 2 * BT],
                        bias1[:, m : m + 1],
                    )

        # -------------------------------------------------------- phase C
        k2A = big.tile([128, 2 * L * B], BF16, tag="kA2")
        k2B = big.tile([128, 2 * L * B], BF16, tag="kB2")
        s2A = big.tile([128, 2 * 2 * B], BF16, tag="sA2")
        s2B = big.tile([128, 2 * 2 * B], BF16, tag="sB2")

        def inj1(m, tau, ps_ap, stop):
            # layer-1 chunk r at step tau needs pre1 at global index
            # L*r + tau - W; in pre1's (EX-headed) chunk frame that is
            # chunk col r + q2, slice u2, with L*q2 + u2 = tau - W + EX*L.
            sh = tau - W + EX * L
            q2, u2 = sh // L, sh % L
            nc.tensor.matmul(
                ps_ap, eye[:, :],
                pre1s[:, m * L * BT + u2 * BT + q2 :][:, :B],
                start=True, stop=stop,
            )

        def reset1(c):
            for scr, off in ((s2A, 4), (s2B, 12)):
                ap = scr[:, c * B : c * B + 2 * B + 1 : 2 * B]
                nc.vector.tensor_tensor(ap, ap, cm[:, off : off + 2],
                                        mybir.AluOpType.mult)
                nc.vector.tensor_tensor(ap, ap, h0r[:, off : off + 2],
                                        mybir.AluOpType.add)

        # output GEMM, batched over 4 kept timesteps (adjacent t-slices are
        # contiguous in the kept tiles, so one matmul covers 4*B columns);
        # emitted as soon as step W+t+3's tanh lands and interleaved into
        # recurrence dependency stalls by the scheduler.
        def emit_out(t):
            if t % 4 != 3:
                return
            t0 = t - 3
            po = psp.tile([128, 1024], F32, name=f"po_{t0}", tag="psA")
            for k in range(4):
                kt = k2A if k < 2 else k2B
                nc.tensor.matmul(
                    po[0:4, : 4 * B],
                    wfc[:, 4 * k : 4 * k + 4],
                    kt[:, (k % 2) * L * B + t0 * B :][:, : 4 * B],
                    start=(k == 0), stop=(k == 3),
                )
            og = outp.tile([3, 4 * B], F32, name=f"og_{t0}", tag="og")
            nc.scalar.activation(og[:], po[0:3, : 4 * B], ACT.Identity,
                                 bias=bfc[:, 0:1])
            nc.sync.dma_start(out_d[:, t0 : t0 + 4, :], og[:])

        _recurrence(nc, psp, whh1, inj1, (k2A, k2B), (s2A, s2B), B, reset1,
                    post=emit_out)

        if DEBUG:
            nc.sync.dma_start(dk1A_d[:], k1A[:])
            nc.sync.dma_start(dk1B_d[:], k1B[:])
            nc.sync.dma_start(dpre_d[:], pre1s[:])
            nc.sync.dma_start(dk2A_d[:], k2A[:])
            nc.sync.dma_start(dk2B_d[:], k2B[:])

    if SPLIT_WAITS:
        _split_sync_waits(nc, maxw=1)
    return nc


_PROG = None


def _pack_lhsT(Wm):
    """[H,H] weight -> [128, 2048] packed stationary tiles: col 512k+128m+j
    holds W^T[128k+p, 128m+j]."""
    Wt = np.ascontiguousarray(Wm.T.astype(np.float32))
    packed = np.zeros((128, 2048), np.float32)
    for k in range(4):
        for m in range(4):
            packed[:, 512 * k + 128 * m : 512 * k + 128 * m + 128] = \
                Wt[128 * k : 128 * k + 128, 128 * m : 128 * m + 128]
    return packed.astype(NPBF16)


def kernel(x, h0, W_ih0, W_hh0, b_ih0, b_hh0, W_ih1, W_hh1, b_ih1, b_hh1,
           W_fc, b_fc):
    global _PROG, LAST_RESULT
    x = np.asarray(x, np.float32)
    h0 = np.asarray(h0, np.float32)

    if _PROG is None:
        _PROG = _build_program()
    nc = _PROG

    w0x = np.zeros((128, 512), np.float32)
    w0x[:IN] = np.asarray(W_ih0, np.float32).T
    w0x[IN] = np.asarray(b_ih0, np.float32) + np.asarray(b_hh0, np.float32)
    w0x = w0x.astype(NPBF16)
    whh0 = _pack_lhsT(np.asarray(W_hh0, np.float32))
    whh1 = _pack_lhsT(np.asarray(W_hh1, np.float32))
    wih1 = _pack_lhsT(np.asarray(W_ih1, np.float32))
    bias1 = (np.asarray(b_ih1, np.float32) + np.asarray(b_hh1, np.float32)) \
        .reshape(4, 128).T.copy()
    wfc = np.zeros((128, 16), np.float32)
    Wfct = np.asarray(W_fc, np.float32).T
    for k in range(4):
        wfc[:, 4 * k : 4 * k + 3] = Wfct[128 * k : 128 * k + 128, :]
    wfc = wfc.astype(NPBF16)
    bfc = np.asarray(b_fc, np.float32).reshape(3, 1)
    eye = np.eye(128, dtype=NPBF16)

    # x^T step-major slabs: xt[i, u*XW + v] = xpad[L*v + u, i]
    head = EX * L + W
    xpad = np.concatenate([np.zeros((head, IN), np.float32), x,
                           np.zeros((L * XW, IN), np.float32)], axis=0)
    in_maps = []
    for p in range(NC):
        s = p * TC
        xs = xpad[s : s + L * XW]                   # [L*XW, IN]
        xsm = xs.reshape(XW, L, IN).transpose(2, 1, 0)  # [IN, L, XW]
        xt = np.zeros((128, L * XW), np.float32)
        xt[:IN] = xsm.reshape(IN, L * XW)
        xt[IN] = 1.0
        h0r = np.zeros((128, 16), np.float32)
        cmv = np.ones((128, 16), np.float32)
        if p == 0:
            cmv[:] = 0.0
            for layer in range(2):
                hk = h0[layer].reshape(4, 128).T  # [128, 4] k-blocks
                # kernel reads: layer0 A=cols 0:2 B=cols 8:10;
                #               layer1 A=cols 4:6 B=cols 12:14
                h0r[:, 4 * layer + 0 : 4 * layer + 2] = hk[:, 0:2]
                h0r[:, 4 * layer + 8 : 4 * layer + 10] = hk[:, 2:4]
        hcm = np.concatenate([h0r, cmv], axis=1).astype(NPBF16)
        in_maps.append({
            "xt": xt.astype(NPBF16), "w0x": w0x, "whh0": whh0, "whh1": whh1,
            "wih1": wih1, "bias1": bias1, "wfc": wfc, "bfc": bfc, "eye": eye,
            "hcm": hcm,
        })

    res = run_bass_kernel_spmd(nc, in_maps, core_ids=list(range(NC)),
                               trace=TRACE)
    LAST_RESULT = res
    out = np.concatenate(
        [res.results[p]["out"].transpose(2, 1, 0).reshape(TC, 3)
         for p in range(NC)], axis=0)
    return out[None, ...].astype(np.float32)


# revision 35
# speedup vs baseline: 1.1123x; 1.1123x over previous
"""Trainium2 Bass kernel for a 2-layer tanh RNN (H=512) over T=32768, batch 1.

Strategy: chunked sequence parallelism with warmup, in bf16. The RNN map
h_t = tanh(pre_t + W_hh h_{t-1}) is contractive (spectral radius ~0.64), so
a chunk recurrence started W steps early from a zero state converges to the
true trajectory; the bf16 quantization noise (~3e-3 rel) dominates the
warmup residual, so W=6 suffices. We split T into 1024 chunks of L=32; each
of the 8 cores advances its 130 chunk-states (128 real + 2 warmup heads) as
one batched recurrence: each step is a [512,512] x [512,130] matmul block
(16 PE tiles, bf16 + automatic fast-weight-load) plus 4 fused input-
injection matmuls and 2 tanh ops. Chunk 0's state is reset to the true h0
after its warmup, making it exact up to bf16 rounding.

Phases per core (fully SPMD, no cross-core communication):
  A) layer-0 batched recurrence (input proj fused as matmuls from x^T;
     w0x zero-padded to K=128 so its weight loads get FWL)
  B) batched GEMM pre1 = W_ih1 @ h1 + bias over the core's time range,
     two timesteps per matmul (adjacent t-slices are contiguous in the
     kept tiles); bias added on the Vector engine (psum f32 -> bf16 SBUF)
  C) layer-1 batched recurrence (pre1 injected via identity matmuls), with
     the output GEMM out = W_fc @ h2 + b_fc emitted per 4 kept timesteps
     (one [4,512] psum group) as soon as they land; the tile scheduler
     interleaves B/D matmuls into recurrence dependency stalls.

bf16 rationale: fp32r needs a >=256-wide moving operand for full rate, which
forces L=16 and a 2x warmup overhead (W=16). bf16 runs 1 col/cycle at any
width and halves weight-load time (automatic FWL), so L=32/W=6 cuts the
per-timestep PE columns by ~30% and the DMA bytes by 2x. PSUM layout: each
of the 8 banks holds at most one open accumulation group, so each m-group
gets its own 512-col bank slice ([128,1024] tiles); phase B/D psum shares
the psA/psB slot rotation. Measured: 256us (fp32r baseline) -> ~168us,
end-to-end rel error 6.6e-3 (vs 2e-2 budget).
"""

import numpy as np
import ml_dtypes

import concourse.bass as bass
import concourse.mybir as mybir
from concourse.tile import TileContext
from concourse.bass_utils import run_bass_kernel_spmd

# ---------------------------------------------------------------- constants
T = 32768
H = 512
IN = 40
NC = 8
L = 32          # chunk length
W = 6           # warmup steps
EX = 2          # extra head chunks per core (W <= EX*L)
TC = T // NC    # timesteps per core
B = TC // L     # real chunks per core
BT = B + EX     # batched chunks per core (layer 0)
XW = BT + 2     # x^T slab width
S = L + W       # recurrence steps per layer
BF16 = mybir.dt.bfloat16
F32 = mybir.dt.float32
ACT = mybir.ActivationFunctionType
NPBF16 = ml_dtypes.bfloat16

TRACE = False        # set by test harness for profiled runs
DEBUG = False        # add intermediate-state DRAM dumps to the program
SPLIT_WAITS = True   # walrus maxw=1 workaround (disable for CoreSim)
LAST_RESULT = None

_ctr = [0]


def _split_sync_waits(nc, maxw=1):
    """walrus in this container encodes at most `maxw` sem-waits per
    instruction; move excess waits onto same-engine NOPs inserted right
    before the instruction (engine program order keeps semantics)."""
    for f in nc.m.functions:
        for bb in f.blocks:
            il = bb.instructions
            targets = []
            for idx, inst in enumerate(il):
                si = inst.sync_info
                if si is not None and si.on_wait is not None and len(si.on_wait) > maxw:
                    targets.append(idx)
            for idx in reversed(targets):
                inst = il[idx]
                si = inst.sync_info
                waits = list(si.on_wait)
                excess = waits[:-maxw]
                inst.sync_info = mybir.SyncInfo(
                    on_wait=waits[-maxw:], on_update=list(si.on_update)
                )
                nops = []
                for j in range(0, len(excess), maxw):
                    _ctr[0] += 1
                    nop = mybir.InstNoOp(name=f"wsplit_nop_{_ctr[0]}")
                    nop.engine = inst.engine
                    nop.sync_info = mybir.SyncInfo(
                        on_wait=excess[j : j + maxw], on_update=[]
                    )
                    nops.append(nop)
                for k, nop in enumerate(nops):
                    il.insert(idx + k, nop)
    return nc


def _recurrence(nc, psp, whh, inject, kept, scr, n, reset, post=None):
    """S batched recurrence steps for one layer.

    kept: (keptA, keptB) flat tiles [128, 2*L*n], col = kh*(L*n) + t*n + b.
    scr:  (scrA, scrB) flat tiles [128, 2*2*n], col = kh*(2*n) + c*n + b.
    inject(m, tau, ps_ap, stop): opens the psum group for output block m.
    reset(c): chunk-0 h0 override hook on scratch ping-pong column c.
    post(t): called after the ACTs of step W+t (t in [0, L)).
    """
    def h_src(k, tp):
        """Contiguous moving AP [128, n] for k-block state after step tp."""
        kh = k % 2
        if tp >= W:
            return kept[k // 2][:, kh * L * n + (tp - W) * n :][:, :n]
        return scr[k // 2][:, kh * 2 * n + (tp % 2) * n :][:, :n]

    def h_dst(a, tau):
        """ACT dst AP [128, 2, n] for half a at step tau."""
        if tau >= W:
            c = tau - W
            return kept[a][:].rearrange("p (k t b) -> p k t b", k=2, t=L)[:, :, c, :]
        c = tau % 2
        return scr[a][:].rearrange("p (k t b) -> p k t b", k=2, t=2)[:, :, c, :]

    for tau in range(S):
        # [128,1024] = 2 PSUM banks; each m-group gets its own bank (one
        # open accumulation group per bank is a hardware constraint).
        psA = psp.tile([128, 1024], F32, name=f"psA_{_ctr[0]}_{tau}", tag="psA")
        psB = psp.tile([128, 1024], F32, name=f"psB_{_ctr[0]}_{tau}", tag="psB")
        ps = (psA, psB)

        def mm_ap(m):
            return ps[m // 2][:, 512 * (m % 2) : 512 * (m % 2) + n]

        # Emit ALL writers of the psA half (m0,m1) before any psB-only work,
        # so psA closes at 10/20 matmuls and the psA-close -> tanh ->
        # next-step-gate chain hides entirely under the psB half's matmuls
        # (the per-step chain is what paces the recurrence).
        for mh in range(2):
            ms = (2 * mh, 2 * mh + 1)
            for m in ms:
                inject(m, tau, mm_ap(m), tau == 0)
            if tau > 0:
                for k in (0, 1):
                    for m in ms:
                        nc.tensor.matmul(
                            mm_ap(m),
                            whh[:, 512 * k + 128 * m : 512 * k + 128 * m + 128],
                            h_src(k, tau - 1),
                            start=False, stop=False,
                        )
                for m in ms:
                    for k in (2, 3):
                        nc.tensor.matmul(
                            mm_ap(m),
                            whh[:, 512 * k + 128 * m : 512 * k + 128 * m + 128],
                            h_src(k, tau - 1),
                            start=False, stop=(k == 3),
                        )
            nc.scalar.activation(
                h_dst(mh, tau),
                ps[mh][:].rearrange("p (m c) -> p m c", m=2)[:, :, :n],
                ACT.Tanh,
            )
        if reset is not None and tau == W - 1:
            reset((W - 1) % 2)
        if post is not None and tau >= W:
            post(tau - W)


def _build_program():
    nc = bass.Bass()
    xt_d = nc.dram_tensor("xt", [128, L * XW], BF16, kind="ExternalInput")
    w0x_d = nc.dram_tensor("w0x", [128, 512], BF16, kind="ExternalInput")
    whh0_d = nc.dram_tensor("whh0", [128, 2048], BF16, kind="ExternalInput")
    whh1_d = nc.dram_tensor("whh1", [128, 2048], BF16, kind="ExternalInput")
    wih1_d = nc.dram_tensor("wih1", [128, 2048], BF16, kind="ExternalInput")
    bias1_d = nc.dram_tensor("bias1", [128, 4], F32, kind="ExternalInput")
    wfc_d = nc.dram_tensor("wfc", [128, 16], BF16, kind="ExternalInput")
    bfc_d = nc.dram_tensor("bfc", [3, 1], F32, kind="ExternalInput")
    eye_d = nc.dram_tensor("eye", [128, 128], BF16, kind="ExternalInput")
    hcm_d = nc.dram_tensor("hcm", [128, 32], BF16, kind="ExternalInput")
    out_d = nc.dram_tensor("out", [3, L, B], F32, kind="ExternalOutput")
    if DEBUG:
        dk1A_d = nc.dram_tensor("dk1A", [128, 2 * L * BT], BF16, kind="ExternalOutput")
        dk1B_d = nc.dram_tensor("dk1B", [128, 2 * L * BT], BF16, kind="ExternalOutput")
        dpre_d = nc.dram_tensor("dpre", [128, 4 * L * BT], BF16, kind="ExternalOutput")
        dk2A_d = nc.dram_tensor("dk2A", [128, 2 * L * B], BF16, kind="ExternalOutput")
        dk2B_d = nc.dram_tensor("dk2B", [128, 2 * L * B], BF16, kind="ExternalOutput")

    import contextlib
    with TileContext(nc) as tc, contextlib.ExitStack() as ctx:
        const = ctx.enter_context(tc.tile_pool(name="const", bufs=1))
        big = ctx.enter_context(tc.tile_pool(name="big", bufs=1))
        outp = ctx.enter_context(tc.tile_pool(name="outp", bufs=2))
        psp = ctx.enter_context(tc.tile_pool(name="psp", bufs=2, space="PSUM"))

        # Critical-path DMAs (gate the first recurrence steps). Each
        # dma_start costs ~600ns of issuing-engine time, so x streams in as
        # 4 big blocks, not 32 slabs. The scalar queue is kept nearly free:
        # ScE runs the per-step tanh.
        xt = const.tile([128, L * XW], BF16)
        w0x = const.tile([128, 512], BF16)
        whh0 = const.tile([128, 2048], BF16)
        hcm = const.tile([128, 32], BF16)
        # block 0 is tiny (2 slabs) so the first step's gate clears fast;
        # whh0 (needed at step 1) is split 4 ways across the three queues
        nc.sync.dma_start(xt[:, : 2 * XW], xt_d[:, : 2 * XW])
        nc.scalar.dma_start(w0x[:], w0x_d[:])
        nc.gpsimd.dma_start(whh0[:, :512], whh0_d[:, :512])
        nc.gpsimd.dma_start(whh0[:, 512:1024], whh0_d[:, 512:1024])
        nc.sync.dma_start(whh0[:, 1024:1536], whh0_d[:, 1024:1536])
        nc.scalar.dma_start(whh0[:, 1536:], whh0_d[:, 1536:])
        nc.scalar.dma_start(hcm[:], hcm_d[:])
        h0r = hcm[:, 0:16]
        cm = hcm[:, 16:32]
        # remaining x blocks stream in behind the recurrence
        qs = (nc.sync, nc.gpsimd, nc.sync)
        for blk, (u0, u1) in enumerate([(2, 12), (12, 22), (22, 32)]):
            qs[blk].dma_start(xt[:, u0 * XW : u1 * XW],
                              xt_d[:, u0 * XW : u1 * XW])
        whh1 = const.tile([128, 2048], BF16)
        wih1 = const.tile([128, 2048], BF16)
        bias1 = const.tile([128, 4], F32)
        wfc = const.tile([128, 16], BF16)
        bfc = const.tile([3, 1], F32)
        eye = const.tile([128, 128], BF16)

        # -------------------------------------------------------- phase A
        k1A = big.tile([128, 2 * L * BT], BF16, tag="kA")
        k1B = big.tile([128, 2 * L * BT], BF16, tag="kB")
        s1A = big.tile([128, 2 * 2 * BT], BF16, tag="sA")
        s1B = big.tile([128, 2 * 2 * BT], BF16, tag="sB")

        def inj0(m, tau, ps_ap, stop):
            # w0x is zero-padded to K=128 so the weight load gets FWL
            q, u = tau // L, tau % L
            nc.tensor.matmul(
                ps_ap, w0x[:, 128 * m : 128 * m + 128],
                xt[:, u * XW + q :][:, :BT],
                start=True, stop=stop,
            )

        def reset0(c):
            for scr, off in ((s1A, 0), (s1B, 8)):
                ap = scr[:, c * BT + EX : c * BT + EX + 2 * BT + 1 : 2 * BT]
                nc.vector.tensor_tensor(ap, ap, cm[:, off : off + 2],
                                        mybir.AluOpType.mult)
                nc.vector.tensor_tensor(ap, ap, h0r[:, off : off + 2],
                                        mybir.AluOpType.add)

        # later-phase weight DMAs: the queues drain them while the PE runs
        # layer 0 (emitted before phase A because phase B consumes them
        # mid-phase-A now).
        nc.gpsimd.dma_start(wih1[:], wih1_d[:])
        nc.sync.dma_start(whh1[:], whh1_d[:])
        nc.sync.dma_start(bias1[:], bias1_d[:])
        nc.gpsimd.dma_start(wfc[:], wfc_d[:])
        nc.gpsimd.dma_start(bfc[:], bfc_d[:])
        nc.sync.dma_start(eye[:], eye_d[:])

        # -------------------------------------------------------- phase B
        # pre1 = W_ih1 @ h1 + bias, step-major: col = m*(L*BT) + t*BT + b
        # (same indexing as h1 kept). Emitted INSIDE phase A via the post
        # hook, one adjacent-t pair-group (2*BT-col matmuls) as soon as both
        # kept timesteps land: this places the psum groups in the right pool-
        # rotation slots so the scheduler can weave B matmuls into phase A's
        # tanh-latency windows. Bias added on Vector (psum f32 -> bf16 SBUF).
        pre1s = big.tile([128, 4 * L * BT], BF16, tag="pre1")

        def emit_b(t):
            if t % 2 != 1:
                return
            t0 = t - 1
            for mp in range(2):
                pg = psp.tile([128, 1024], F32, name=f"pg_{t0}_{mp}", tag="psB")
                for mi in range(2):
                    m = 2 * mp + mi
                    for k in range(4):
                        kt = k1A if k < 2 else k1B
                        nc.tensor.matmul(
                            pg[:, 512 * mi : 512 * mi + 2 * BT],
                            wih1[:, 512 * k + 128 * m : 512 * k + 128 * m + 128],
                            kt[:, (k % 2) * L * BT + t0 * BT :][:, : 2 * BT],
                            start=(k == 0), stop=(k == 3),
                        )
                for mi in range(2):
                    m = 2 * mp + mi
                    nc.vector.tensor_scalar_add(
                        pre1s[:, m * L * BT + t0 * BT :][:, : 2 * BT],
                        pg[:, 512 * mi : 512 * mi + 2 * BT],
                        bias1[:, m : m + 1],
                    )

        _recurrence(nc, psp, whh0, inj0, (k1A, k1B), (s1A, s1B), BT, reset0,
                    post=emit_b)

        # -------------------------------------------------------- phase C
        k2A = big.tile([128, 2 * L * B], BF16, tag="kA2")
        k2B = big.tile([128, 2 * L * B], BF16, tag="kB2")
        s2A = big.tile([128, 2 * 2 * B], BF16, tag="sA2")
        s2B = big.tile([128, 2 * 2 * B], BF16, tag="sB2")

        def inj1(m, tau, ps_ap, stop):
            # layer-1 chunk r at step tau needs pre1 at global index
            # L*r + tau - W; in pre1's (EX-headed) chunk frame that is
            # chunk col r + q2, slice u2, with L*q2 + u2 = tau - W + EX*L.
            sh = tau - W + EX * L
            q2, u2 = sh // L, sh % L
            nc.tensor.matmul(
                ps_ap, eye[:, :],
                pre1s[:, m * L * BT + u2 * BT + q2 :][:, :B],
                start=True, stop=stop,
            )

        def reset1(c):
            for scr, off in ((s2A, 4), (s2B, 12)):
                ap = scr[:, c * B : c * B + 2 * B + 1 : 2 * B]
                nc.vector.tensor_tensor(ap, ap, cm[:, off : off + 2],
                                        mybir.AluOpType.mult)
                nc.vector.tensor_tensor(ap, ap, h0r[:, off : off + 2],
                                        mybir.AluOpType.add)

        # output GEMM, batched over 4 kept timesteps (adjacent t-slices are
        # contiguous in the kept tiles, so one matmul covers 4*B columns);
        # emitted as soon as step W+t+3's tanh lands and interleaved into
        # recurrence dependency stalls by the scheduler.
        def emit_out(t):
            if t % 4 != 3:
                return
            t0 = t - 3
            po = psp.tile([128, 1024], F32, name=f"po_{t0}", tag="psA")
            for k in range(4):
                kt = k2A if k < 2 else k2B
                nc.tensor.matmul(
                    po[0:4, : 4 * B],
                    wfc[:, 4 * k : 4 * k + 4],
                    kt[:, (k % 2) * L * B + t0 * B :][:, : 4 * B],
                    start=(k == 0), stop=(k == 3),
                )
            og = outp.tile([3, 4 * B], F32, name=f"og_{t0}", tag="og")
            nc.scalar.activation(og[:], po[0:3, : 4 * B], ACT.Identity,
                                 bias=bfc[:, 0:1])
            nc.sync.dma_start(out_d[:, t0 : t0 + 4, :], og[:])

        _recurrence(nc, psp, whh1, inj1, (k2A, k2B), (s2A, s2B), B, reset1,
                    post=emit_out)

        if DEBUG:
            nc.sync.dma_start(dk1A_d[:], k1A[:])
            nc.sync.dma_start(dk1B_d[:], k1B[:])
            nc.sync.dma_start(dpre_d[:], pre1s[:])
            nc.sync.dma_start(dk2A_d[:], k2A[:])
            nc.sync.dma_start(dk2B_d[:], k2B[:])

    if SPLIT_WAITS:
        _split_sync_waits(nc, maxw=1)
    return nc


_PROG = None


def _pack_lhsT(Wm):
    """[H,H] weight -> [128, 2048] packed stationary tiles: col 512k+128m+j
    holds W^T[128k+p, 128m+j]."""
    Wt = np.ascontiguousarray(Wm.T.astype(np.float32))
    packed = np.zeros((128, 2048), np.float32)
    for k in range(4):
        for m in range(4):
            packed[:, 512 * k + 128 * m : 512 * k + 128 * m + 128] = \
                Wt[128 * k : 128 * k + 128, 128 * m : 128 * m + 128]
    return packed.astype(NPBF16)


def kernel(x, h0, W_ih0, W_hh0, b_ih0, b_hh0, W_ih1, W_hh1, b_ih1, b_hh1,
           W_fc, b_fc):
    global _PROG, LAST_RESULT
    x = np.asarray(x, np.float32)
    h0 = np.asarray(h0, np.float32)

    if _PROG is None:
        _PROG = _build_program()
    nc = _PROG

    w0x = np.zeros((128, 512), np.float32)
    w0x[:IN] = np.asarray(W_ih0, np.float32).T
    w0x[IN] = np.asarray(b_ih0, np.float32) + np.asarray(b_hh0, np.float32)
    w0x = w0x.astype(NPBF16)
    whh0 = _pack_lhsT(np.asarray(W_hh0, np.float32))
    whh1 = _pack_lhsT(np.asarray(W_hh1, np.float32))
    wih1 = _pack_lhsT(np.asarray(W_ih1, np.float32))
    bias1 = (np.asarray(b_ih1, np.float32) + np.asarray(b_hh1, np.float32)) \
        .reshape(4, 128).T.copy()
    wfc = np.zeros((128, 16), np.float32)
    Wfct = np.asarray(W_fc, np.float32).T
    for k in range(4):
        wfc[:, 4 * k : 4 * k + 3] = Wfct[128 * k : 128 * k + 128, :]
    wfc = wfc.astype(NPBF16)
    bfc = np.asarray(b_fc, np.float32).reshape(3, 1)
    eye = np.eye(128, dtype=NPBF16)

    # x^T step-major slabs: xt[i, u*XW + v] = xpad[L*v + u, i]
    head = EX * L + W
    xpad = np.concatenate([np.zeros((head, IN), np.float32), x,
                           np.zeros((L * XW, IN), np.float32)], axis=0)
    in_maps = []
    for p in range(NC):
        s = p * TC
        xs = xpad[s : s + L * XW]                   # [L*XW, IN]
        xsm = xs.reshape(XW, L, IN).transpose(2, 1, 0)  # [IN, L, XW]
        xt = np.zeros((128, L * XW), np.float32)
        xt[:IN] = xsm.reshape(IN, L * XW)
        xt[IN] = 1.0
        h0r = np.zeros((128, 16), np.float32)
        cmv = np.ones((128, 16), np.float32)
        if p == 0:
            cmv[:] = 0.0
            for layer in range(2):
                hk = h0[layer].reshape(4, 128).T  # [128, 4] k-blocks
                # kernel reads: layer0 A=cols 0:2 B=cols 8:10;
                #               layer1 A=cols 4:6 B=cols 12:14
                h0r[:, 4 * layer + 0 : 4 * layer + 2] = hk[:, 0:2]
                h0r[:, 4 * layer + 8 : 4 * layer + 10] = hk[:, 2:4]
        hcm = np.concatenate([h0r, cmv], axis=1).astype(NPBF16)
        in_maps.append({
            "xt": xt.astype(NPBF16), "w0x": w0x, "whh0": whh0, "whh1": whh1,
            "wih1": wih1, "bias1": bias1, "wfc": wfc, "bfc": bfc, "eye": eye,
            "hcm": hcm,
        })

    res = run_bass_kernel_spmd(nc, in_maps, core_ids=list(range(NC)),
                               trace=TRACE)
    LAST_RESULT = res
    out = np.concatenate(
        [res.results[p]["out"].transpose(2, 1, 0).reshape(TC, 3)
         for p in range(NC)], axis=0)
    return out[None, ...].astype(np.float32)


# revision 36
# speedup vs baseline: 1.1558x; 1.0392x over previous
"""Trainium2 Bass kernel for a 2-layer tanh RNN (H=512) over T=32768, batch 1.

Strategy: chunked sequence parallelism with warmup, in bf16. The RNN map
h_t = tanh(pre_t + W_hh h_{t-1}) is contractive (spectral radius ~0.64), so
a chunk recurrence started W steps early from a zero state converges to the
true trajectory; the bf16 quantization noise (~3e-3 rel) dominates the
warmup residual, so W=6 suffices. We split T into 1024 chunks of L=32; each
of the 8 cores advances its 130 chunk-states (128 real + 2 warmup heads) as
one batched recurrence: each step is a [512,512] x [512,130] matmul block
(16 PE tiles, bf16 + automatic fast-weight-load) plus 4 fused input-
injection matmuls and 2 tanh ops. Chunk 0's state is reset to the true h0
after its warmup, making it exact up to bf16 rounding.

Phases per core (fully SPMD, no cross-core communication):
  A) layer-0 batched recurrence (input proj fused as matmuls from x^T;
     w0x zero-padded to K=128 so its weight loads get FWL)
  B) batched GEMM pre1 = W_ih1 @ h1 + bias over the core's time range,
     two timesteps per matmul (adjacent t-slices are contiguous in the
     kept tiles); bias added on the Vector engine (psum f32 -> bf16 SBUF)
  C) layer-1 batched recurrence (pre1 injected via identity matmuls), with
     the output GEMM out = W_fc @ h2 + b_fc emitted per 4 kept timesteps
     (one [4,512] psum group) as soon as they land; the tile scheduler
     interleaves B/D matmuls into recurrence dependency stalls.

bf16 rationale: fp32r needs a >=256-wide moving operand for full rate, which
forces L=16 and a 2x warmup overhead (W=16). bf16 runs 1 col/cycle at any
width and halves weight-load time (automatic FWL), so L=32/W=6 cuts the
per-timestep PE columns by ~30% and the DMA bytes by 2x. PSUM layout: each
of the 8 banks holds at most one open accumulation group, so each m-group
gets its own 512-col bank slice ([128,1024] tiles); phase B/D psum shares
the psA/psB slot rotation. Measured: 256us (fp32r baseline) -> ~168us,
end-to-end rel error 6.6e-3 (vs 2e-2 budget).
"""

import numpy as np
import ml_dtypes

import concourse.bass as bass
import concourse.mybir as mybir
from concourse.tile import TileContext
from concourse.bass_utils import run_bass_kernel_spmd

# ---------------------------------------------------------------- constants
T = 32768
H = 512
IN = 40
NC = 8
L = 32          # chunk length
W = 6           # warmup steps
EX = 2          # extra head chunks per core (W <= EX*L)
TC = T // NC    # timesteps per core
B = TC // L     # real chunks per core
BT = B + EX     # batched chunks per core (layer 0)
XW = BT + 2     # x^T slab width
S = L + W       # recurrence steps per layer
BF16 = mybir.dt.bfloat16
F32 = mybir.dt.float32
ACT = mybir.ActivationFunctionType
NPBF16 = ml_dtypes.bfloat16

TRACE = False        # set by test harness for profiled runs
DEBUG = False        # add intermediate-state DRAM dumps to the program
SPLIT_WAITS = True   # walrus maxw=1 workaround (disable for CoreSim)
LAST_RESULT = None

_ctr = [0]


def _split_sync_waits(nc, maxw=1):
    """walrus in this container encodes at most `maxw` sem-waits per
    instruction; move excess waits onto same-engine NOPs inserted right
    before the instruction (engine program order keeps semantics)."""
    for f in nc.m.functions:
        for bb in f.blocks:
            il = bb.instructions
            targets = []
            for idx, inst in enumerate(il):
                si = inst.sync_info
                if si is not None and si.on_wait is not None and len(si.on_wait) > maxw:
                    targets.append(idx)
            for idx in reversed(targets):
                inst = il[idx]
                si = inst.sync_info
                waits = list(si.on_wait)
                excess = waits[:-maxw]
                inst.sync_info = mybir.SyncInfo(
                    on_wait=waits[-maxw:], on_update=list(si.on_update)
                )
                nops = []
                for j in range(0, len(excess), maxw):
                    _ctr[0] += 1
                    nop = mybir.InstNoOp(name=f"wsplit_nop_{_ctr[0]}")
                    nop.engine = inst.engine
                    nop.sync_info = mybir.SyncInfo(
                        on_wait=excess[j : j + maxw], on_update=[]
                    )
                    nops.append(nop)
                for k, nop in enumerate(nops):
                    il.insert(idx + k, nop)
    return nc


def _recurrence(nc, psp, whh, inject, kept, scr, n, reset, post=None):
    """S batched recurrence steps for one layer.

    kept: (keptA, keptB) flat tiles [128, 2*L*n], col = kh*(L*n) + t*n + b.
    scr:  (scrA, scrB) flat tiles [128, 2*2*n], col = kh*(2*n) + c*n + b.
    inject(m, tau, ps_ap, stop): opens the psum group for output block m.
    reset(c): chunk-0 h0 override hook on scratch ping-pong column c.
    post(t): called after the ACTs of step W+t (t in [0, L)).
    """
    def h_src(k, tp):
        """Contiguous moving AP [128, n] for k-block state after step tp."""
        kh = k % 2
        if tp >= W:
            return kept[k // 2][:, kh * L * n + (tp - W) * n :][:, :n]
        return scr[k // 2][:, kh * 2 * n + (tp % 2) * n :][:, :n]

    def h_dst(a, tau):
        """ACT dst AP [128, 2, n] for half a at step tau."""
        if tau >= W:
            c = tau - W
            return kept[a][:].rearrange("p (k t b) -> p k t b", k=2, t=L)[:, :, c, :]
        c = tau % 2
        return scr[a][:].rearrange("p (k t b) -> p k t b", k=2, t=2)[:, :, c, :]

    for tau in range(S):
        # [128,1024] = 2 PSUM banks; each m-group gets its own bank (one
        # open accumulation group per bank is a hardware constraint).
        psA = psp.tile([128, 1024], F32, name=f"psA_{_ctr[0]}_{tau}", tag="psA")
        psB = psp.tile([128, 1024], F32, name=f"psB_{_ctr[0]}_{tau}", tag="psB")
        ps = (psA, psB)

        def mm_ap(m):
            return ps[m // 2][:, 512 * (m % 2) : 512 * (m % 2) + n]

        # Emit ALL writers of the psA half (m0,m1) before any psB-only work,
        # so psA closes at 10/20 matmuls and the psA-close -> tanh ->
        # next-step-gate chain hides entirely under the psB half's matmuls
        # (the per-step chain is what paces the recurrence).
        for mh in range(2):
            ms = (2 * mh, 2 * mh + 1)
            for m in ms:
                inject(m, tau, mm_ap(m), tau == 0)
            if tau > 0:
                for k in (0, 1):
                    for m in ms:
                        nc.tensor.matmul(
                            mm_ap(m),
                            whh[:, 512 * k + 128 * m : 512 * k + 128 * m + 128],
                            h_src(k, tau - 1),
                            start=False, stop=False,
                        )
                for m in ms:
                    for k in (2, 3):
                        nc.tensor.matmul(
                            mm_ap(m),
                            whh[:, 512 * k + 128 * m : 512 * k + 128 * m + 128],
                            h_src(k, tau - 1),
                            start=False, stop=(k == 3),
                        )
            nc.scalar.activation(
                h_dst(mh, tau),
                ps[mh][:].rearrange("p (m c) -> p m c", m=2)[:, :, :n],
                ACT.Tanh,
            )
        if reset is not None and tau == W - 1:
            reset((W - 1) % 2)
        if post is not None and tau >= W:
            post(tau - W)


def _build_program():
    nc = bass.Bass()
    xt_d = nc.dram_tensor("xt", [128, L * XW], BF16, kind="ExternalInput")
    w0x_d = nc.dram_tensor("w0x", [128, 512], BF16, kind="ExternalInput")
    whh0_d = nc.dram_tensor("whh0", [128, 2048], BF16, kind="ExternalInput")
    whh1_d = nc.dram_tensor("whh1", [128, 2048], BF16, kind="ExternalInput")
    wih1_d = nc.dram_tensor("wih1", [128, 2048], BF16, kind="ExternalInput")
    bias1_d = nc.dram_tensor("bias1", [128, 4], F32, kind="ExternalInput")
    wfc_d = nc.dram_tensor("wfc", [128, 16], BF16, kind="ExternalInput")
    bfc_d = nc.dram_tensor("bfc", [3, 1], F32, kind="ExternalInput")
    eye_d = nc.dram_tensor("eye", [128, 128], BF16, kind="ExternalInput")
    hcm_d = nc.dram_tensor("hcm", [128, 32], BF16, kind="ExternalInput")
    out_d = nc.dram_tensor("out", [3, L, B], F32, kind="ExternalOutput")
    if DEBUG:
        dk1A_d = nc.dram_tensor("dk1A", [128, 2 * L * BT], BF16, kind="ExternalOutput")
        dk1B_d = nc.dram_tensor("dk1B", [128, 2 * L * BT], BF16, kind="ExternalOutput")
        dpre_d = nc.dram_tensor("dpre", [128, 4 * L * BT], BF16, kind="ExternalOutput")
        dk2A_d = nc.dram_tensor("dk2A", [128, 2 * L * B], BF16, kind="ExternalOutput")
        dk2B_d = nc.dram_tensor("dk2B", [128, 2 * L * B], BF16, kind="ExternalOutput")

    import contextlib
    with TileContext(nc) as tc, contextlib.ExitStack() as ctx:
        const = ctx.enter_context(tc.tile_pool(name="const", bufs=1))
        big = ctx.enter_context(tc.tile_pool(name="big", bufs=1))
        outp = ctx.enter_context(tc.tile_pool(name="outp", bufs=2))
        psp = ctx.enter_context(tc.tile_pool(name="psp", bufs=2, space="PSUM"))

        # Critical-path DMAs (gate the first recurrence steps). Each
        # dma_start costs ~600ns of issuing-engine time, so x streams in as
        # 4 big blocks, not 32 slabs. The scalar queue is kept nearly free:
        # ScE runs the per-step tanh.
        xt = const.tile([128, L * XW], BF16)
        w0x = const.tile([128, 512], BF16)
        whh0 = const.tile([128, 2048], BF16)
        hcm = const.tile([128, 32], BF16)
        # block 0 is tiny (2 slabs) so the first step's gate clears fast;
        # whh0 (needed at step 1) is split 4 ways across the three queues
        nc.sync.dma_start(xt[:, : 2 * XW], xt_d[:, : 2 * XW])
        nc.scalar.dma_start(w0x[:], w0x_d[:])
        nc.gpsimd.dma_start(whh0[:, :512], whh0_d[:, :512])
        nc.gpsimd.dma_start(whh0[:, 512:1024], whh0_d[:, 512:1024])
        nc.sync.dma_start(whh0[:, 1024:1536], whh0_d[:, 1024:1536])
        nc.scalar.dma_start(whh0[:, 1536:], whh0_d[:, 1536:])
        nc.scalar.dma_start(hcm[:], hcm_d[:])
        h0r = hcm[:, 0:16]
        cm = hcm[:, 16:32]
        # remaining x blocks stream in behind the recurrence
        qs = (nc.sync, nc.gpsimd, nc.sync)
        for blk, (u0, u1) in enumerate([(2, 12), (12, 22), (22, 32)]):
            qs[blk].dma_start(xt[:, u0 * XW : u1 * XW],
                              xt_d[:, u0 * XW : u1 * XW])
        whh1 = const.tile([128, 2048], BF16)
        wih1 = const.tile([128, 2048], BF16)
        bias1 = const.tile([128, 4], F32)
        wfc = const.tile([128, 16], BF16)
        bfc = const.tile([3, 1], F32)
        eye = const.tile([128, 128], BF16)

        # -------------------------------------------------------- phase A
        k1A = big.tile([128, 2 * L * BT], BF16, tag="kA")
        k1B = big.tile([128, 2 * L * BT], BF16, tag="kB")
        s1A = big.tile([128, 2 * 2 * BT], BF16, tag="sA")
        s1B = big.tile([128, 2 * 2 * BT], BF16, tag="sB")

        def inj0(m, tau, ps_ap, stop):
            # w0x is zero-padded to K=128 so the weight load gets FWL
            q, u = tau // L, tau % L
            nc.tensor.matmul(
                ps_ap, w0x[:, 128 * m : 128 * m + 128],
                xt[:, u * XW + q :][:, :BT],
                start=True, stop=stop,
            )

        def reset0(c):
            for scr, off in ((s1A, 0), (s1B, 8)):
                ap = scr[:, c * BT + EX : c * BT + EX + 2 * BT + 1 : 2 * BT]
                nc.vector.tensor_tensor(ap, ap, cm[:, off : off + 2],
                                        mybir.AluOpType.mult)
                nc.vector.tensor_tensor(ap, ap, h0r[:, off : off + 2],
                                        mybir.AluOpType.add)

        # later-phase weight DMAs: the queues drain them while the PE runs
        # layer 0 (emitted before phase A because phase B consumes them
        # mid-phase-A now).
        nc.gpsimd.dma_start(wih1[:], wih1_d[:])
        nc.sync.dma_start(whh1[:], whh1_d[:])
        nc.sync.dma_start(bias1[:], bias1_d[:])
        nc.gpsimd.dma_start(wfc[:], wfc_d[:])
        nc.gpsimd.dma_start(bfc[:], bfc_d[:])
        nc.sync.dma_start(eye[:], eye_d[:])

        # -------------------------------------------------------- phase B
        # pre1 = W_ih1 @ h1 + bias, step-major: col = m*(L*BT) + t*BT + b
        # (same indexing as h1 kept). Emitted INSIDE phase A via the post
        # hook, one adjacent-t pair-group (2*BT-col matmuls) as soon as both
        # kept timesteps land: this places the psum groups in the right pool-
        # rotation slots so the scheduler can weave B matmuls into phase A's
        # tanh-latency windows. Bias added on Vector (psum f32 -> bf16 SBUF).
        pre1s = big.tile([128, 4 * L * BT], BF16, tag="pre1")

        def emit_b(t):
            # ONE pg tile per pair-group (two allocations would flip the psB
            # slot parity and put our Vector read on phase A's psum path);
            # the two m-pairs run as sequential groups in the same banks.
            if t % 2 != 1:
                return
            t0 = t - 1
            pg = psp.tile([128, 1024], F32, name=f"pg_{t0}", tag="psB")
            for mp in range(2):
                for mi in range(2):
                    m = 2 * mp + mi
                    for k in range(4):
                        kt = k1A if k < 2 else k1B
                        nc.tensor.matmul(
                            pg[:, 512 * mi : 512 * mi + 2 * BT],
                            wih1[:, 512 * k + 128 * m : 512 * k + 128 * m + 128],
                            kt[:, (k % 2) * L * BT + t0 * BT :][:, : 2 * BT],
                            start=(k == 0), stop=(k == 3),
                        )
                for mi in range(2):
                    m = 2 * mp + mi
                    nc.vector.tensor_scalar_add(
                        pre1s[:, m * L * BT + t0 * BT :][:, : 2 * BT],
                        pg[:, 512 * mi : 512 * mi + 2 * BT],
                        bias1[:, m : m + 1],
                    )

        _recurrence(nc, psp, whh0, inj0, (k1A, k1B), (s1A, s1B), BT, reset0,
                    post=emit_b)

        # -------------------------------------------------------- phase C
        k2A = big.tile([128, 2 * L * B], BF16, tag="kA2")
        k2B = big.tile([128, 2 * L * B], BF16, tag="kB2")
        s2A = big.tile([128, 2 * 2 * B], BF16, tag="sA2")
        s2B = big.tile([128, 2 * 2 * B], BF16, tag="sB2")

        def inj1(m, tau, ps_ap, stop):
            # layer-1 chunk r at step tau needs pre1 at global index
            # L*r + tau - W; in pre1's (EX-headed) chunk frame that is
            # chunk col r + q2, slice u2, with L*q2 + u2 = tau - W + EX*L.
            sh = tau - W + EX * L
            q2, u2 = sh // L, sh % L
            nc.tensor.matmul(
                ps_ap, eye[:, :],
                pre1s[:, m * L * BT + u2 * BT + q2 :][:, :B],
                start=True, stop=stop,
            )

        def reset1(c):
            for scr, off in ((s2A, 4), (s2B, 12)):
                ap = scr[:, c * B : c * B + 2 * B + 1 : 2 * B]
                nc.vector.tensor_tensor(ap, ap, cm[:, off : off + 2],
                                        mybir.AluOpType.mult)
                nc.vector.tensor_tensor(ap, ap, h0r[:, off : off + 2],
                                        mybir.AluOpType.add)

        # output GEMM, batched over 4 kept timesteps (adjacent t-slices are
        # contiguous in the kept tiles, so one matmul covers 4*B columns);
        # emitted as soon as step W+t+3's tanh lands and interleaved into
        # recurrence dependency stalls by the scheduler.
        def emit_out(t):
            if t % 4 != 3:
                return
            t0 = t - 3
            po = psp.tile([128, 1024], F32, name=f"po_{t0}", tag="psA")
            for k in range(4):
                kt = k2A if k < 2 else k2B
                nc.tensor.matmul(
                    po[0:4, : 4 * B],
                    wfc[:, 4 * k : 4 * k + 4],
                    kt[:, (k % 2) * L * B + t0 * B :][:, : 4 * B],
                    start=(k == 0), stop=(k == 3),
                )
            og = outp.tile([3, 4 * B], F32, name=f"og_{t0}", tag="og")
            nc.scalar.activation(og[:], po[0:3, : 4 * B], ACT.Identity,
                                 bias=bfc[:, 0:1])
            nc.sync.dma_start(out_d[:, t0 : t0 + 4, :], og[:])

        _recurrence(nc, psp, whh1, inj1, (k2A, k2B), (s2A, s2B), B, reset1,
                    post=emit_out)

        if DEBUG:
            nc.sync.dma_start(dk1A_d[:], k1A[:])
            nc.sync.dma_start(dk1B_d[:], k1B[:])
            nc.sync.dma_start(dpre_d[:], pre1s[:])
            nc.sync.dma_start(dk2A_d[:], k2A[:])
            nc.sync.dma_start(dk2B_d[:], k2B[:])

    if SPLIT_WAITS:
        _split_sync_waits(nc, maxw=1)
    return nc


_PROG = None


def _pack_lhsT(Wm):
    """[H,H] weight -> [128, 2048] packed stationary tiles: col 512k+128m+j
    holds W^T[128k+p, 128m+j]."""
    Wt = np.ascontiguousarray(Wm.T.astype(np.float32))
    packed = np.zeros((128, 2048), np.float32)
    for k in range(4):
        for m in range(4):
            packed[:, 512 * k + 128 * m : 512 * k + 128 * m + 128] = \
                Wt[128 * k : 128 * k + 128, 128 * m : 128 * m + 128]
    return packed.astype(NPBF16)


def kernel(x, h0, W_ih0, W_hh0, b_ih0, b_hh0, W_ih1, W_hh1, b_ih1, b_hh1,
           W_fc, b_fc):
    global _PROG, LAST_RESULT
    x = np.asarray(x, np.float32)
    h0 = np.asarray(h0, np.float32)

    if _PROG is None:
        _PROG = _build_program()
    nc = _PROG

    w0x = np.zeros((128, 512), np.float32)
    w0x[:IN] = np.asarray(W_ih0, np.float32).T
    w0x[IN] = np.asarray(b_ih0, np.float32) + np.asarray(b_hh0, np.float32)
    w0x = w0x.astype(NPBF16)
    whh0 = _pack_lhsT(np.asarray(W_hh0, np.float32))
    whh1 = _pack_lhsT(np.asarray(W_hh1, np.float32))
    wih1 = _pack_lhsT(np.asarray(W_ih1, np.float32))
    bias1 = (np.asarray(b_ih1, np.float32) + np.asarray(b_hh1, np.float32)) \
        .reshape(4, 128).T.copy()
    wfc = np.zeros((128, 16), np.float32)
    Wfct = np.asarray(W_fc, np.float32).T
    for k in range(4):
        wfc[:, 4 * k : 4 * k + 3] = Wfct[128 * k : 128 * k + 128, :]
    wfc = wfc.astype(NPBF16)
    bfc = np.asarray(b_fc, np.float32).reshape(3, 1)
    eye = np.eye(128, dtype=NPBF16)

    # x^T step-major slabs: xt[i, u*XW + v] = xpad[L*v + u, i]
    head = EX * L + W
    xpad = np.concatenate([np.zeros((head, IN), np.float32), x,
                           np.zeros((L * XW, IN), np.float32)], axis=0)
    in_maps = []
    for p in range(NC):
        s = p * TC
        xs = xpad[s : s + L * XW]                   # [L*XW, IN]
        xsm = xs.reshape(XW, L, IN).transpose(2, 1, 0)  # [IN, L, XW]
        xt = np.zeros((128, L * XW), np.float32)
        xt[:IN] = xsm.reshape(IN, L * XW)
        xt[IN] = 1.0
        h0r = np.zeros((128, 16), np.float32)
        cmv = np.ones((128, 16), np.float32)
        if p == 0:
            cmv[:] = 0.0
            for layer in range(2):
                hk = h0[layer].reshape(4, 128).T  # [128, 4] k-blocks
                # kernel reads: layer0 A=cols 0:2 B=cols 8:10;
                #               layer1 A=cols 4:6 B=cols 12:14
                h0r[:, 4 * layer + 0 : 4 * layer + 2] = hk[:, 0:2]
                h0r[:, 4 * layer + 8 : 4 * layer + 10] = hk[:, 2:4]
        hcm = np.concatenate([h0r, cmv], axis=1).astype(NPBF16)
        in_maps.append({
            "xt": xt.astype(NPBF16), "w0x": w0x, "whh0": whh0, "whh1": whh1,
            "wih1": wih1, "bias1": bias1, "wfc": wfc, "bfc": bfc, "eye": eye,
            "hcm": hcm,
        })

    res = run_bass_kernel_spmd(nc, in_maps, core_ids=list(range(NC)),
                               trace=TRACE)
    LAST_RESULT = res
    out = np.concatenate(
        [res.results[p]["out"].transpose(2, 1, 0).reshape(TC, 3)
         for p in range(NC)], axis=0)
    return out[None, ...].astype(np.float32)
